# revision 51
# baseline (speedup 1.0000x reference)
"""Trainium2 Bass kernel for BinaryNormalizedLinear.

Computes (data-parallel over the token dim across 8 NeuronCores):
    W_q = (W > mean(W)).astype(f32)          # global mean over all of W
    b_q = (b > mean(b)).astype(f32)
    z   = x @ W_q.T + b_q                    # [M, OUT]
    out = (z - mean(z, -1)) / (sqrt(var(z, -1, ddof=1)) + 1e-8)

Distribution / dataflow:
  * x is split along M across cores (normalization is row-local, so no
    output collective).
  * W's global mean: each core partial-sums its own OUT/8 row-slice,
    followed by a tiny AllReduce.  (A full W_q AllGather was tried and
    measured ~250us on this fabric - collectives here run at cross-chip
    bandwidth - so each core instead streams + binarizes all of W locally,
    pipelined under the GEMM.)
  * GEMM: stationary = x, moving = W_q in fp8 ({0,1} exact).  Most of x is
    bf16; NDR of the 16 contraction-block pairs run as fp8 DoubleRow
    (2 MACs/cell/cycle), which cuts PE time ~25% while keeping the
    measured rel err at 1.5e-2 (< 2e-2 budget; bf16-only is 4e-3).
  * z for o-tiles 0..OT-2 round-trips through DRAM in bf16; the last
    o-tile is computed m-block-outer and normalized straight from PSUM so
    normalize/store overlaps the remaining matmuls.
"""

import os
from contextlib import ExitStack

import ml_dtypes
import numpy as np

P = 128
NF = 512  # output columns per o-tile (one PSUM bank of fp32)
EPS = 1e-8
NDR = 6   # of the IB//2 contraction-block pairs, how many run as fp8 DoubleRow


class Cfg:
    def __init__(self, n_cores, M, IN, OUT, ndr=NDR):
        self.n_cores = n_cores
        self.M = M
        self.IN = IN
        self.OUT = OUT
        self.M_LOC = M // n_cores        # rows of x per core
        self.MB = self.M_LOC // P        # m blocks per core
        self.IB = IN // P                # contraction blocks
        self.OT = OUT // NF              # output column tiles
        self.OTC = self.OT // n_cores    # o-tiles in the per-core mean slice
        self.WSLF = self.IB * NF         # per-partition elems of one o-tile of W^T
        npairs = self.IB // 2
        ndr = min(ndr, npairs)
        # fp8 DoubleRow pairs (ib blocks 2j, 2j+1), spread over the range
        self.dr_pairs = sorted({int(i * npairs / ndr) for i in range(ndr)}) if ndr else []
        dr_blocks = set()
        for j in self.dr_pairs:
            dr_blocks.add(2 * j)
            dr_blocks.add(2 * j + 1)
        self.dr_blocks = dr_blocks


FULL = Cfg(8, 8192, 4096, 4096)


def emit(ctx, tc, cfg, xT, wslT, Wt, b_in, out_t, pfx=""):
    """Emit the kernel body into TileContext tc.

    xT:   [IB, P, M_LOC] f32        per-core x^T, i on partitions
    wslT: [OTC, IB, P, NF] f32      own o-slice(s) of W^T (for the mean)
    Wt:   [OT, IB, P, NF] f32       full W^T (streamed + binarized locally)
    b_in: [OUT] f32
    out_t:[MB, P, OUT] f32          per-core output rows (m = mb*128 + p)
    """
    import concourse.bass as bass
    import concourse.mybir as mybir
    from concourse import bass_isa
    from concourse.bass import _add_dep_helper

    nc = tc.nc
    f32 = mybir.dt.float32
    bf16 = mybir.dt.bfloat16
    fp8 = mybir.dt.float8e4
    Alu = mybir.AluOpType

    CH = min(4096, cfg.WSLF)     # W stream chunk free-size (per partition)
    NCH = cfg.WSLF // CH
    IBC = CH // NF               # ib blocks per chunk

    def dep(after, before, why):
        _add_dep_helper(after.ins, before.ins, sync=True, reason=why)

    # contraction-block schedule: DR pairs cover (2j, 2j+1), the rest bf16.
    # Also the x-load emission order, so arrival tracks consumption.
    sched = []
    ib = 0
    while ib < cfg.IB:
        if ib in cfg.dr_blocks:
            sched.append(("dr", ib))
            ib += 2
        else:
            sched.append(("bf", ib))
            ib += 1

    # ---- pools ----
    small = ctx.enter_context(tc.tile_pool(name=pfx + "small", bufs=4))
    singles = ctx.enter_context(tc.tile_pool(name=pfx + "singles", bufs=1))
    dram = ctx.enter_context(tc.tile_pool(name=pfx + "dram", bufs=1, space="DRAM"))
    psum_pool = ctx.enter_context(tc.tile_pool(name=pfx + "psum", bufs=8, space="PSUM"))
    zio = ctx.enter_context(tc.tile_pool(name=pfx + "zio", bufs=3))
    # W f32 stream chunks and binarized W_q chunks (per-chunk-slot double
    # buffering via bufs)
    wstream = ctx.enter_context(tc.tile_pool(name=pfx + "wst", bufs=2))
    wqpool = ctx.enter_context(tc.tile_pool(name=pfx + "wqp", bufs=3))
    # the per-core mean slice stream is only needed in the head
    wslpool = tc.alloc_tile_pool(name=pfx + "wsl", bufs=2)

    # ---- persistent SBUF tiles ----
    x_sb = {}
    for ib in range(cfg.IB):
        if ib not in cfg.dr_blocks:
            x_sb[ib] = singles.tile(
                [P, cfg.M_LOC], bf16, tag=f"x{ib}", name=f"{pfx}x{ib}"
            )
    x8_sb = {}
    for j in cfg.dr_pairs:
        x8_sb[j] = singles.tile(
            [P, 2, cfg.M_LOC], fp8, tag=f"x8_{j}", name=f"{pfx}x8_{j}"
        )
    stats_mb = [
        singles.tile([P, cfg.OT, 6], f32, tag=f"stats{mb}", name=f"{pfx}stats{mb}")
        for mb in range(cfg.MB)
    ]
    bq_sb = singles.tile([P, cfg.OUT], bf16, tag="bq_sb", name=pfx + "bq_sb")
    mu_bcast = singles.tile([P, 1], f32, tag="mu_bcast", name=pfx + "mu_bcast")

    # z for o-tiles 0..OT-3 round-trips DRAM; the last TWO o-tiles finalize
    # straight from PSUM (8 banks = 4 m-blocks x 2 o-tiles per group)
    NZT = max(cfg.OT - 2, 0)
    Z_W = NZT * NF
    z_dram = (
        dram.tile([cfg.MB, P, Z_W], bf16, name=pfx + "z_dram") if Z_W else None
    )

    # ---- W global mean: per-core partial sum over own slice + AllReduce ----
    nparts = cfg.OTC * NCH
    wm_parts = singles.tile([P, nparts], f32, tag="wm_parts", name=pfx + "wm_parts")
    wsl_last_ld = None
    for k in range(cfg.OTC):
        for j in range(NCH):
            t = wslpool.tile([P, CH], f32, tag="wslst", name=f"{pfx}wsls{k}_{j}")
            wsl_last_ld = nc.scalar.dma_start(
                t.rearrange("p (b f) -> p b f", b=IBC),
                wslT[k, j * IBC : (j + 1) * IBC].rearrange("b p f -> p b f"),
            )
            nc.vector.tensor_reduce(
                wm_parts[:, k * NCH + j : k * NCH + j + 1],
                t,
                axis=mybir.AxisListType.X,
                op=Alu.add,
            )
    wm_red = small.tile([P, 1], f32, tag="wm_red", name=pfx + "wm_red")
    nc.vector.tensor_reduce(wm_red, wm_parts, axis=mybir.AxisListType.X, op=Alu.add)
    wsum_bc = small.tile([P, 1], f32, tag="wsum_bc", name=pfx + "wsum_bc")
    nc.gpsimd.partition_all_reduce(
        wsum_bc, wm_red, channels=P, reduce_op=bass_isa.ReduceOp.add
    )

    # Bounce DMAs on scalar/HWDGE; the x8 SWDGE loads are ordered after the
    # trigger (below) so they cannot delay it on the gpsimd queue.
    ar_in = dram.tile([P, 1], f32, name=pfx + "ar_in")
    ar_out = dram.tile([P, 1], f32, name=pfx + "ar_out")
    ar_wr = nc.scalar.dma_start(ar_in[:], wsum_bc)
    if cfg.n_cores > 1:
        ar = nc.gpsimd.collective_compute(
            "AllReduce",
            Alu.add,
            replica_groups=[list(range(cfg.n_cores))],
            ins=[ar_in.opt()],
            outs=[ar_out.opt()],
        )
    else:
        ar = nc.scalar.dma_start(ar_out[:], ar_in[:])
    dep(ar, ar_wr, "AR reads ar_in")
    mu_raw = small.tile([P, 1], f32, tag="mu_raw", name=pfx + "mu_raw")
    ar_rd = nc.scalar.dma_start(mu_raw, ar_out[:])
    dep(ar_rd, ar, "mu read waits for AllReduce")
    nc.scalar.mul(mu_bcast, mu_raw, 1.0 / float(cfg.OUT * cfg.IN))

    # ---- x loads (bf16 from host; DR pairs recast to fp8 via SWDGE).
    # Deferred behind the mean-slice stream so the head DMA is not starved.
    for kind, ib in sched:
        if kind == "bf":
            eng = nc.sync if ib % 2 == 0 else nc.scalar
            ld = eng.dma_start(x_sb[ib], xT[ib])
            dep(ld, wsl_last_ld, "x after the mean-slice stream")
        else:
            ld = nc.gpsimd.dma_start(
                x8_sb[ib // 2], xT[ib : ib + 2].rearrange("b p m -> p b m")
            )
            dep(ld, ar, "x8 SWDGE after the AR trigger on the gpsimd queue")

    # ---- b quantization (tiny; off the critical path) ----
    BF = cfg.OUT // P
    b_pt = singles.tile([P, BF], f32, tag="b_pt", name=pfx + "b_pt")
    nc.scalar.dma_start(b_pt, b_in.rearrange("(p f) -> p f", p=P))
    bsum = small.tile([P, 1], f32, tag="bsum", name=pfx + "bsum")
    nc.vector.tensor_reduce(bsum, b_pt, axis=mybir.AxisListType.X, op=Alu.add)
    bsum_bc = small.tile([P, 1], f32, tag="bsum_bc", name=pfx + "bsum_bc")
    nc.gpsimd.partition_all_reduce(
        bsum_bc, bsum, channels=P, reduce_op=bass_isa.ReduceOp.add
    )
    bmean = small.tile([P, 1], f32, tag="bmean", name=pfx + "bmean")
    nc.scalar.mul(bmean, bsum_bc, 1.0 / float(cfg.OUT))
    bq_pt = singles.tile([P, BF], bf16, tag="bq_pt", name=pfx + "bq_pt")
    nc.vector.tensor_scalar(bq_pt, b_pt, bmean, None, op0=Alu.is_gt)
    bq_dram = dram.tile([cfg.OUT], bf16, name=pfx + "bq_dram")
    nc.scalar.dma_start(bq_dram.rearrange("(p f) -> p f", p=P), bq_pt)
    nc.scalar.dma_start(bq_sb, bq_dram[None, :].to_broadcast([P, cfg.OUT]))

    # ---- W_q production: stream Wt[ot] chunks, binarize to fp8 ----
    def make_wq(ot):
        """Returns list of NCH fp8 chunk tiles [P, IBC, NF] for o-tile ot."""
        qs = []
        for j in range(NCH):
            st = wstream.tile([P, CH], f32, tag="wst", name=f"{pfx}w{ot}_{j}")
            eng = nc.sync if (ot + j) % 2 == 0 else nc.scalar
            ld = eng.dma_start(
                st.rearrange("p (b f) -> p b f", b=IBC),
                Wt[ot, j * IBC : (j + 1) * IBC].rearrange("b p f -> p b f"),
            )
            if ot <= 1:
                # the first o-tile pair's W is on the critical path right
                # after mu, but it must not steal DMA bandwidth from the
                # mean-slice stream
                dep(ld, wsl_last_ld, "first W pair after the mean-slice stream")
            q = wqpool.tile([P, CH], fp8, tag=f"wq{j}", name=f"{pfx}wq{ot}_{j}")
            nc.vector.tensor_scalar(q, st, mu_bcast, None, op0=Alu.is_gt)
            qs.append(q)
        return qs

    def gemm_mm(wq, psum, mb, entry, first, last):
        kind, ib = entry
        j, r = divmod(ib, IBC)
        if kind == "dr":
            nc.tensor.matmul(
                psum,
                lhsT=x8_sb[ib // 2][:, :, mb * P : (mb + 1) * P],
                rhs=wq[j].rearrange("p (b f) -> p b f", b=IBC)[:, r : r + 2, :],
                start=first,
                stop=last,
                perf_mode=mybir.MatmulPerfMode.DoubleRow,
            )
        else:
            nc.tensor.matmul(
                psum,
                lhsT=x_sb[ib][:, mb * P : (mb + 1) * P],
                rhs=wq[j][:, r * NF : (r + 1) * NF],
                start=first,
                stop=last,
            )

    ddof_scale = float(cfg.OUT) / float(cfg.OUT - 1)

    def drain_mb(ot, mb, psum):
        # z = psum + b_q (bf16), partial row stats, park z in DRAM
        osl = slice(ot * NF, (ot + 1) * NF)
        z_t = zio.tile([P, NF], bf16, tag="zio", name=f"{pfx}zw{ot}_{mb}")
        nc.vector.tensor_tensor(z_t, psum, bq_sb[:, osl], op=Alu.add)
        nc.vector.bn_stats(stats_mb[mb][:, ot, :], z_t)
        nc.scalar.dma_start(z_dram[mb, :, osl], z_t)

    # the mean-slice stream pool has no further readers; free its space for
    # the tail pools (LIFO: wsl was the top of the pool stack)
    wslpool.release()
    zrd = ctx.enter_context(tc.tile_pool(name=pfx + "zrd", bufs=3))
    ostage = ctx.enter_context(tc.tile_pool(name=pfx + "ostage", bufs=6))

    def finalize_mb(mb, ps, otA, otB, z_rt):
        """Row stats + normalize + store for one m-block; the last two
        o-tiles' z comes straight from PSUM (ps dict), the rest from z_rt."""
        for ot in (otA, otB):
            osl = slice(ot * NF, (ot + 1) * NF)
            nc.vector.tensor_tensor(
                ps[(mb, ot)], ps[(mb, ot)], bq_sb[:, osl], op=Alu.add
            )
            nc.vector.bn_stats(stats_mb[mb][:, ot, :], ps[(mb, ot)])
        # (z - mean) / (sqrt(var * n/(n-1)) + eps)
        mv = small.tile([P, 2], f32, tag="mv", name=f"{pfx}mv{mb}")
        nc.vector.bn_aggr(mv, stats_mb[mb])
        std = small.tile([P, 1], f32, tag="std", name=f"{pfx}std{mb}")
        nc.scalar.activation(
            std, mv[:, 1:2], mybir.ActivationFunctionType.Sqrt, scale=ddof_scale
        )
        nc.vector.tensor_scalar_add(std, std, EPS)
        rstd = small.tile([P, 1], f32, tag="rstd", name=f"{pfx}rstd{mb}")
        nc.vector.reciprocal(rstd, std)
        # normalize on the (otherwise idle) ACT engine:
        # out = z*rstd + (-mean*rstd)
        negmr = small.tile([P, 1], f32, tag="negmr", name=f"{pfx}negmr{mb}")
        nc.vector.tensor_tensor(negmr, mv[:, 0:1], rstd, op=Alu.mult)
        nc.scalar.mul(negmr, negmr, -1.0)
        # PSUM-sourced slices first: frees the banks for the next group's
        # matmuls without waiting for the z readback
        o2s = [otA, otB] + [o for o in range(cfg.OT) if o not in (otA, otB)]
        for oi, o2 in enumerate(o2s):
            src = ps[(mb, o2)] if o2 in (otA, otB) else (
                z_rt[:, o2 * NF : (o2 + 1) * NF]
            )
            row_t = ostage.tile([P, NF], f32, tag="o_t", name=f"{pfx}o{mb}_{o2}")
            nc.scalar.activation(
                row_t,
                src,
                mybir.ActivationFunctionType.Identity,
                bias=negmr,
                scale=rstd,
            )
            eng = [nc.sync, nc.scalar, nc.gpsimd][(mb + oi) % 3]
            eng.dma_start(out_t[mb][:, o2 * NF : (o2 + 1) * NF], row_t)

    # ---- GEMM: o-tile PAIRS x m-block groups of 4 -> 8 PSUM banks.
    # Within a (pair, group) every stationary x tile feeds two consecutive
    # matmuls (both o-tiles), halving LDWEIGHTS pressure.  The final pair
    # skips the z round-trip and finalizes straight from PSUM, each group's
    # normalize/store overlapping the next group's matmuls.
    assert cfg.OT % 2 == 0
    GRP = min(4, cfg.MB)
    NPR = cfg.OT // 2
    wqs = {0: make_wq(0), 1: make_wq(1)}
    for pr in range(NPR):
        otA, otB = 2 * pr, 2 * pr + 1
        last = pr == NPR - 1
        wqA, wqB = wqs.pop(otA), wqs.pop(otB)
        if last and cfg.MB >= 8:
            # shrink the trailing groups: less finalize work exposed after
            # the very last matmul
            groups = [(0, 4), (4, 2), (6, 1), (7, 1)]
        else:
            groups = [(g0, GRP) for g0 in range(0, cfg.MB, GRP)]
        for g0, gn in groups:
            mbs = range(g0, min(g0 + gn, cfg.MB))
            z_rts = {}
            if last and Z_W:
                for mb in mbs:
                    z_rt = zrd.tile([P, Z_W], bf16, tag="zrd", name=f"{pfx}zrd{mb}")
                    (nc.scalar if mb % 2 == 0 else nc.gpsimd).dma_start(
                        z_rt, z_dram[mb]
                    )
                    z_rts[mb] = z_rt
            ps = {
                (mb, ot): psum_pool.tile(
                    [P, NF], f32, tag="ps", name=f"{pfx}ps{ot}_{mb}"
                )
                for mb in mbs
                for ot in (otA, otB)
            }
            for si, entry in enumerate(sched):
                for mb in mbs:
                    for ot in (otA, otB):
                        gemm_mm(
                            wqA if ot == otA else wqB,
                            ps[(mb, ot)],
                            mb,
                            entry,
                            si == 0,
                            si == len(sched) - 1,
                        )
            if g0 == 0 and not last:
                # prefetch the next pair's W stream behind this pair
                wqs[otA + 2] = make_wq(otA + 2)
                wqs[otB + 2] = make_wq(otB + 2)
            for mb in mbs:
                if last:
                    finalize_mb(mb, ps, otA, otB, z_rts.get(mb))
                else:
                    drain_mb(otA, mb, ps[(mb, otA)])
                    drain_mb(otB, mb, ps[(mb, otB)])


def _io_tensors(nc, cfg):
    import concourse.mybir as mybir

    f32 = mybir.dt.float32
    bf16 = mybir.dt.bfloat16
    xT = nc.dram_tensor("xT", [cfg.IB, P, cfg.M_LOC], bf16, kind="ExternalInput").ap()
    wslT = nc.dram_tensor(
        "wslT", [cfg.OTC, cfg.IB, P, NF], f32, kind="ExternalInput"
    ).ap()
    Wt = nc.dram_tensor(
        "Wt", [cfg.OT, cfg.IB, P, NF], f32, kind="ExternalInput"
    ).ap()
    b_in = nc.dram_tensor("b_in", [cfg.OUT], f32, kind="ExternalInput").ap()
    out_t = nc.dram_tensor(
        "out", [cfg.MB, P, cfg.OUT], f32, kind="ExternalOutput"
    ).ap()
    return xT, wslT, Wt, b_in, out_t


def build(cfg):
    import concourse.tile as tile
    from concourse import bacc

    nc = bacc.Bacc(
        "TRN2",
        target_bir_lowering=False,
        debug=False,
        num_devices=cfg.n_cores,
    )
    xT, wslT, Wt, b_in, out_t = _io_tensors(nc, cfg)
    with tile.TileContext(nc) as tc:
        with ExitStack() as ctx:
            emit(ctx, tc, cfg, xT, wslT, Wt, b_in, out_t)
    nc.compile()
    return nc


def build_repeat(cfg, reps):
    """Build a variant executing the whole kernel `reps` times back-to-back
    (same I/O tensors), for slope-based device timing."""
    import concourse.tile as tile
    from concourse import bacc

    nc = bacc.Bacc(
        "TRN2",
        target_bir_lowering=False,
        debug=False,
        num_devices=cfg.n_cores,
    )
    xT, wslT, Wt, b_in, out_t = _io_tensors(nc, cfg)
    with tile.TileContext(nc) as tc:
        for r in range(reps):
            with ExitStack() as ctx:
                emit(ctx, tc, cfg, xT, wslT, Wt, b_in, out_t, pfx=f"r{r}_")
    nc.compile()
    return nc


def prep_in_maps(x, W, b, cfg):
    x = np.ascontiguousarray(x, dtype=np.float32)
    W = np.ascontiguousarray(W, dtype=np.float32)
    b = np.ascontiguousarray(b, dtype=np.float32)
    # Wt[ot, ib, p, f] = W[ot*NF + f, ib*P + p]
    Wt = np.ascontiguousarray(W.reshape(cfg.OT, NF, cfg.IB, P).transpose(0, 2, 3, 1))
    in_maps = []
    xb = x.astype(ml_dtypes.bfloat16)  # staging format: the GEMM consumes bf16
    for c in range(cfg.n_cores):
        xc = xb[c * cfg.M_LOC : (c + 1) * cfg.M_LOC]
        xT = np.ascontiguousarray(xc.reshape(cfg.M_LOC, cfg.IB, P).transpose(1, 2, 0))
        wslT = Wt[c * cfg.OTC : (c + 1) * cfg.OTC]
        in_maps.append({"xT": xT, "wslT": wslT, "Wt": Wt, "b_in": b})
    return in_maps


class Runner:
    """Executes a compiled Bass module over 8 cores via PJRT (axon), with
    input staging separated from execution so repeated runs can be timed."""

    def __init__(self, nc, n_cores):
        import jax
        import concourse.mybir as mybir
        from concourse.bass2jax import (
            _bass_exec_p,
            install_neuronx_cc_hook,
            partition_id_tensor,
        )
        from jax.experimental.shard_map import shard_map
        from jax.sharding import Mesh, NamedSharding, PartitionSpec

        install_neuronx_cc_hook()
        self.jax = jax
        self.n_cores = n_cores
        partition_name = (
            nc.partition_id_tensor.name if nc.partition_id_tensor else None
        )
        in_names = []
        out_names = []
        out_avals = []
        self.out_shapes = []
        for alloc in nc.m.functions[0].allocations:
            if not isinstance(alloc, mybir.MemoryLocationSet):
                continue
            name = alloc.memorylocations[0].name
            if alloc.kind == "ExternalInput":
                if name != partition_name:
                    in_names.append(name)
            elif alloc.kind == "ExternalOutput":
                shape = tuple(alloc.tensor_shape)
                dtype = mybir.dt.np(alloc.dtype)
                out_names.append(name)
                out_avals.append(jax.core.ShapedArray(shape, dtype))
                self.out_shapes.append((shape, dtype))
        self.n_params = len(in_names)
        self.in_names = list(in_names)
        self.out_names = list(out_names)
        in_names_full = in_names + out_names
        if partition_name is not None:
            in_names_full.append(partition_name)

        self._p = _bass_exec_p
        self._partition_id_tensor = partition_id_tensor
        self._partition_name = partition_name
        self._bind_kwargs = dict(
            out_avals=tuple(out_avals),
            in_names=tuple(in_names_full),
            out_names=tuple(out_names),
            lowering_input_output_aliases=(),
            sim_require_finite=True,
            sim_require_nnan=True,
            nc=nc,
        )

        def _body(*args):
            operands = list(args)
            if partition_name is not None:
                operands.append(partition_id_tensor())
            outs = _bass_exec_p.bind(*operands, **self._bind_kwargs)
            return tuple(outs)

        devices = jax.devices()[:n_cores]
        assert len(devices) == n_cores
        self.mesh = Mesh(np.asarray(devices), ("core",))
        n_outs = len(out_names)
        in_specs = (PartitionSpec("core"),) * (self.n_params + n_outs)
        out_specs = (PartitionSpec("core"),) * n_outs
        self.fn = jax.jit(
            shard_map(
                _body,
                mesh=self.mesh,
                in_specs=in_specs,
                out_specs=out_specs,
                check_rep=False,
            ),
            keep_unused=True,
        )
        self.sharding = NamedSharding(self.mesh, PartitionSpec("core"))
        self.staged = None

    def stage(self, in_maps):
        jax = self.jax
        concat = [
            np.concatenate([np.asarray(m[name]) for m in in_maps], axis=0)
            for name in self.in_names
        ]
        concat += [
            np.zeros((self.n_cores * s[0], *s[1:]), d) for s, d in self.out_shapes
        ]
        self.staged = [jax.device_put(a, self.sharding) for a in concat]
        self.jax.block_until_ready(self.staged)

    def run(self):
        outs = self.fn(*self.staged)
        self.jax.block_until_ready(outs)
        return {
            name: np.asarray(outs[i]).reshape(
                self.n_cores, *self.out_shapes[i][0]
            )
            for i, name in enumerate(self.out_names)
        }

    def timeit(self, iters=20):
        import time

        ts = []
        for _ in range(iters):
            t0 = time.perf_counter()
            outs = self.fn(*self.staged)
            self.jax.block_until_ready(outs)
            ts.append(time.perf_counter() - t0)
        return ts


_cache = {}


def get_runner():
    cfg = FULL
    if "runner" not in _cache:
        _cache["runner"] = Runner(build(cfg), cfg.n_cores)
    return _cache["runner"]


def kernel(x, W, b):
    cfg = FULL
    assert x.shape == (cfg.M, cfg.IN) and W.shape == (cfg.OUT, cfg.IN)
    r = get_runner()
    r.stage(prep_in_maps(x, W, b, cfg))
    outs = r.run()
    out = outs["out"].reshape(cfg.n_cores * cfg.MB, P, cfg.OUT).reshape(
        cfg.M, cfg.OUT
    )
    return np.ascontiguousarray(out, dtype=np.float32)


kernel.last_exec_ns = None
